# revision 28
# baseline (speedup 1.0000x reference)
"""DOTA mix E-step (vq_codebook) on 8 TRN2 NeuronCores.

out[b,k,m] = gamma_class[b,k] * softmax_m(-0.5*(log_det+maha) + log_pi)

Math: logits'[b,j] = sum_d x2[b,d]*W1[d,j] + sum_d x[b,d]*W2[d,j]
with W1 = -0.5/var + carry (legal fold since sum_d x^2 = 1), W2 = mu/var,
  carry[j] = -0.5*(log_det + mu'inv mu) + log_pi - max_valid_m(-0.5*log_det + log_pi)
so logits' <= 0 and softmax needs no max pass:
  out = gamma * exp(logits') / group_sum(exp(logits'))

Mask-aware packing: modes beyond a class's count are padding (output exactly
0), so only valid modes enter the GEMM. Classes are bucketed by mode count;
bucket sizes are rounded to a multiple of 8 by promoting classes from the
next-lower count pool (1 wasted column each) so all cores compile one
identical SPMD program. Leftover count-1 classes are handled on host
(softmax of one element is exactly 1, so out = gamma there). The packed
per-core output is scattered into the zeroed (B, K, M) tensor on host.
"""

import sys

import numpy as np

sys.path.insert(0, "/opt/trn_rl_repo")

import concourse.bass as bass
import concourse.mybir as mybir
import concourse.tile as tile
from concourse import bacc, bass_utils

F32 = mybir.dt.float32
F16 = mybir.dt.float16

B, K, M, D = 4096, 1000, 8, 512
NCORES = 8
NB = B // 128             # 32 batch chunks
EPS_REG = 1e-3


def build_bass(buckets):
    """buckets: tuple of (width, n_classes_per_core) for widths 2..8."""
    nv = sum(w * n for w, n in buckets)       # packed logit columns per core
    kc = sum(n for _, n in buckets)           # packed classes per core
    # column tiles: one per PSUM bank (matmul output must stay in-bank,
    # bank-aligned)
    colt = [(0, min(512, nv))] + ([(512, nv - 512)] if nv > 512 else [])
    assert nv <= 1024

    nc = bacc.Bacc("TRN2", debug=False, target_bir_lowering=False)
    xt = nc.dram_tensor("xt", (NB, 4, 128, 128), F16, kind="ExternalInput")
    w1 = nc.dram_tensor("w1", (D, nv), F16, kind="ExternalInput")
    w2 = nc.dram_tensor("w2", (D, nv), F16, kind="ExternalInput")
    gam = nc.dram_tensor("gam", (128, NB * kc), F32, kind="ExternalInput")
    out = nc.dram_tensor("out", (B, nv), F32, kind="ExternalOutput")
    warm = nc.dram_tensor("warm", (128, 128), F32, kind="ExternalOutput")

    xt_ap, w1_ap, w2_ap, gam_ap, out_ap = (
        xt.ap(), w1.ap(), w2.ap(), gam.ap(), out.ap())

    # bcast-muls on GPSIMD except the widest bucket (DVE has slack)
    widest = max(range(len(buckets)), key=lambda i: buckets[i][0] * buckets[i][1])
    gp_buckets = set(range(len(buckets))) - {widest}

    with tile.TileContext(nc) as tc:
        with (
            tc.tile_pool(name="wpool", bufs=1) as wpool,
            tc.tile_pool(name="xpool", bufs=4) as xpool,
            tc.tile_pool(name="ppool", bufs=4, space="PSUM") as ppool,
            tc.tile_pool(name="epool", bufs=5) as epool,
            tc.tile_pool(name="spool", bufs=5) as spool,
            tc.tile_pool(name="opool", bufs=5) as opool,
        ):
            w1t, w2t = [], []
            for r in range(4):
                t = wpool.tile([128, nv], F16, tag=f"w1_{r}")
                nc.sync.dma_start(t[:], w1_ap[r * 128:(r + 1) * 128, :])
                w1t.append(t)
            for r in range(4):
                t = wpool.tile([128, nv], F16, tag=f"w2_{r}")
                nc.sync.dma_start(t[:], w2_ap[r * 128:(r + 1) * 128, :])
                w2t.append(t)
            gall = wpool.tile([128, NB * kc], F32, tag="gam")
            nc.sync.dma_start(gall[:], gam_ap[:, :])

            # HAM warmup: ~5us of dummy matmuls while DMAs land, so the
            # real GEMM starts at 2.4 GHz instead of 1.2
            wz = wpool.tile([128, 128], F16, tag="warmz")
            nc.vector.memset(wz[:], 0.0)
            wps = ppool.tile([128, 1024], F32, tag="ps")
            for i in range(36):
                nc.tensor.matmul(wps[:, 0:128], lhsT=wz[:], rhs=wz[:],
                                 start=True, stop=True)
            wsb = wpool.tile([128, 128], F32, tag="warmsb")
            nc.vector.tensor_copy(wsb[:], wps[:, 0:128])
            nc.sync.dma_start(warm.ap()[:, :], wsb[:])

            for bc in range(NB):
                if bc % 4 == 0:
                    xb4 = xpool.tile([128, 2048], F16, tag="xb")
                    nc.scalar.dma_start(
                        xb4[:].rearrange("p (c r j) -> p c r j", c=4, r=4),
                        xt_ap[bc:bc + 4].rearrange("c r p j -> p c r j"))
                    x2b4 = xpool.tile([128, 2048], F16, tag="x2b")
                    nc.scalar.square(x2b4[:], xb4[:])
                xb = xb4[:, (bc % 4) * 512:(bc % 4) * 512 + 512]
                x2b = x2b4[:, (bc % 4) * 512:(bc % 4) * 512 + 512]

                ps = ppool.tile([128, 1024], F32, tag="ps")
                for c0, cw in colt:
                    for k in range(4):
                        nc.tensor.matmul(
                            ps[:, c0:c0 + cw],
                            lhsT=x2b[:, k * 128:(k + 1) * 128],
                            rhs=w1t[k][:, c0:c0 + cw],
                            start=(k == 0), stop=False,
                        )
                    for k in range(4):
                        nc.tensor.matmul(
                            ps[:, c0:c0 + cw],
                            lhsT=xb[:, k * 128:(k + 1) * 128],
                            rhs=w2t[k][:, c0:c0 + cw],
                            start=False, stop=(k == 3),
                        )

                e = epool.tile([128, nv], F32)
                nc.scalar.activation(e[:], ps[:, 0:nv],
                                     mybir.ActivationFunctionType.Exp)

                ssum = spool.tile([128, kc], F32, tag="ssum")
                off = koff = 0
                for w, n in buckets:
                    nc.vector.reduce_sum(
                        ssum[:, koff:koff + n],
                        e[:, off:off + n * w].rearrange(
                            "p (k m) -> p k m", m=w),
                        axis=mybir.AxisListType.X,
                    )
                    off += n * w
                    koff += n
                rec = spool.tile([128, kc], F32, tag="rec")
                nc.vector.reciprocal_approx_fast(rec[:], ssum[:])
                coef = spool.tile([128, kc], F32, tag="coef")
                nc.vector.tensor_mul(coef[:], rec[:],
                                     gall[:, bc * kc:(bc + 1) * kc])

                o = opool.tile([128, nv], F32)
                off = koff = 0
                for i, (w, n) in enumerate(buckets):
                    e3 = e[:, off:off + n * w].rearrange(
                        "p (k m) -> p k m", m=w)
                    o3 = o[:, off:off + n * w].rearrange(
                        "p (k m) -> p k m", m=w)
                    c3 = coef[:, koff:koff + n].rearrange(
                        "p (k one) -> p k one", one=1)
                    e3b, c3b = bass.broadcast_tensor_aps(e3, c3)
                    eng = nc.gpsimd if i in gp_buckets else nc.vector
                    eng.tensor_tensor(o3, e3b, c3b, op=mybir.AluOpType.mult)
                    off += n * w
                    koff += n
                nc.sync.dma_start(out_ap[bc * 128:(bc + 1) * 128, :], o[:])

    nc.compile()
    return nc


def _layout(mask):
    """Bucket classes by count; round each bucket to a multiple of NCORES by
    promoting classes from the next-lower count pool (cost: 1 wasted column
    each, vs w for a dummy), cascading w=8 down to 2; remaining gaps get
    dummies (-1). Leftover count-1 classes are handled on host (out = gamma
    at mode 0)."""
    counts = mask.sum(-1).astype(int)               # (K,)
    pools = {c: list(np.where(counts == c)[0]) for c in range(1, M + 1)}
    entries = []                                    # (w, global id list)
    for w in range(M, 1, -1):
        ids = pools[w]
        pools[w] = []
        pad = (-len(ids)) % NCORES
        if pad and len(pools[w - 1]) >= pad:
            ids += pools[w - 1][:pad]
            pools[w - 1] = pools[w - 1][pad:]
        elif pad:
            ids += [-1] * pad
        if ids:
            entries.append((w, ids))
    entries.sort()
    per_core = [[] for _ in range(NCORES)]
    buckets = []
    for w, ids in entries:
        n = len(ids) // NCORES
        buckets.append((w, n))
        for c in range(NCORES):
            per_core[c].append((w, ids[c * n:(c + 1) * n]))
    ones = np.asarray(pools[1], int)
    return tuple(buckets), per_core, ones


def prep_inputs(x, gamma_class, mu_pad, var_pad, pi_pad, mask):
    x = np.asarray(x, np.float32)
    gamma_class = np.asarray(gamma_class, np.float32)
    mask = np.asarray(mask, bool)

    var = np.clip(np.asarray(var_pad, np.float64) + EPS_REG, 1e-8, None)
    inv = 1.0 / var
    logdet = np.log(var).sum(-1)                      # (K, M)
    muinv = np.asarray(mu_pad, np.float64) * inv
    muinvmu = (np.asarray(mu_pad, np.float64) * muinv).sum(-1)
    logpi = np.where(mask, np.log(np.asarray(pi_pad, np.float64) + 1e-10),
                     -np.inf)
    u = -0.5 * logdet + logpi
    c = u.max(axis=1)
    carry = np.maximum(u - 0.5 * muinvmu - c[:, None], -20000.0)  # (K, M)

    w1_full = (-0.5 * inv + carry[..., None]).astype(np.float32)  # (K, M, D)
    w2_full = muinv.astype(np.float32)

    buckets, per_core, ones = _layout(mask)
    nv = sum(w * n for w, n in buckets)
    kc = sum(n for _, n in buckets)

    xtb = np.ascontiguousarray(
        x.reshape(NB, 128, 4, 128).transpose(0, 2, 3, 1).astype(np.float16))

    in_maps = []
    for cidx in range(NCORES):
        w1c = np.zeros((nv, D), np.float32)
        w2c = np.zeros((nv, D), np.float32)
        gcols = np.zeros((B, kc), np.float32)
        off = koff = 0
        for w, ids in per_core[cidx]:
            for k in ids:
                if k >= 0:
                    w1c[off:off + w] = w1_full[k, :w]
                    w2c[off:off + w] = w2_full[k, :w]
                    gcols[:, koff] = gamma_class[:, k]
                off += w
                koff += 1
        in_maps.append({
            "xt": xtb,
            "w1": np.ascontiguousarray(w1c.T.astype(np.float16)),
            "w2": np.ascontiguousarray(w2c.T.astype(np.float16)),
            "gam": np.ascontiguousarray(
                gcols.reshape(NB, 128, kc).transpose(1, 0, 2)
                .reshape(128, NB * kc)),
        })
    return in_maps, buckets, per_core, ones


_NC_CACHE = {}


def _get_nc(buckets):
    if buckets not in _NC_CACHE:
        _NC_CACHE[buckets] = build_bass(buckets)
    return _NC_CACHE[buckets]


def kernel(x, gamma_class, mu_pad, var_pad, pi_pad, mask, _trace=False):
    in_maps, buckets, per_core, ones = prep_inputs(
        x, gamma_class, mu_pad, var_pad, pi_pad, mask)
    if not buckets:  # every class has a single mode: softmax is identically 1
        out = np.zeros((B, K, M), np.float32)
        out[:, ones, 0] = np.asarray(gamma_class, np.float32)[:, ones]
        return out
    nc = _get_nc(buckets)
    res = bass_utils.run_bass_kernel_spmd(
        nc, in_maps, core_ids=list(range(NCORES)), trace=_trace)

    counts = np.asarray(mask, bool).sum(-1).astype(int)
    out = np.zeros((B, K, M), np.float32)
    gamma_class = np.asarray(gamma_class, np.float32)
    if len(ones):
        out[:, ones, 0] = gamma_class[:, ones]
    for cidx in range(NCORES):
        packed = res.results[cidx]["out"]             # (B, nv)
        off = 0
        for w, ids in per_core[cidx]:
            ids = np.asarray(ids)
            blk = packed[:, off:off + len(ids) * w].reshape(B, len(ids), w)
            real = ids >= 0
            # promoted classes have count < w: scatter only their real modes
            for c in np.unique(counts[ids[real]]):
                sel = real & (counts[np.maximum(ids, 0)] == c)
                out[:, ids[sel], :c] = blk[:, sel, :c]
            off += len(ids) * w
    if _trace:
        kernel.last_results = res
    return out


# revision 29
# speedup vs baseline: 1.0237x; 1.0237x over previous
"""DOTA mix E-step (vq_codebook) on 8 TRN2 NeuronCores.

out[b,k,m] = gamma_class[b,k] * softmax_m(-0.5*(log_det+maha) + log_pi)

Math: logits'[b,j] = sum_d x2[b,d]*W1[d,j] + sum_d x[b,d]*W2[d,j]
with W1 = -0.5/var + carry (legal fold since sum_d x^2 = 1), W2 = mu/var,
  carry[j] = -0.5*(log_det + mu'inv mu) + log_pi - max_valid_m(-0.5*log_det + log_pi)
so logits' <= 0 and softmax needs no max pass:
  out = gamma * exp(logits') / group_sum(exp(logits'))

Mask-aware packing: modes beyond a class's count are padding (output exactly
0), so only valid modes enter the GEMM. Classes are bucketed by mode count;
bucket sizes are rounded to a multiple of 8 by promoting classes from the
next-lower count pool (1 wasted column each) so all cores compile one
identical SPMD program. Leftover count-1 classes are handled on host
(softmax of one element is exactly 1, so out = gamma there). The packed
per-core output is scattered into the zeroed (B, K, M) tensor on host.
"""

import sys

import numpy as np

sys.path.insert(0, "/opt/trn_rl_repo")

import concourse.bass as bass
import concourse.mybir as mybir
import concourse.tile as tile
from concourse import bacc, bass_utils

F32 = mybir.dt.float32
F16 = mybir.dt.float16

B, K, M, D = 4096, 1000, 8, 512
NCORES = 8
NB = B // 128             # 32 batch chunks
EPS_REG = 1e-3


def build_bass(buckets):
    """buckets: tuple of (width, n_classes_per_core) for widths 2..8."""
    nv = sum(w * n for w, n in buckets)       # packed logit columns per core
    kc = sum(n for _, n in buckets)           # packed classes per core
    # column tiles: one per PSUM bank (matmul output must stay in-bank,
    # bank-aligned)
    colt = [(0, min(512, nv))] + ([(512, nv - 512)] if nv > 512 else [])
    assert nv <= 1024

    nc = bacc.Bacc("TRN2", debug=False, target_bir_lowering=False)
    xt = nc.dram_tensor("xt", (NB, 4, 128, 128), F16, kind="ExternalInput")
    w1 = nc.dram_tensor("w1", (D, nv), F16, kind="ExternalInput")
    w2 = nc.dram_tensor("w2", (D, nv), F16, kind="ExternalInput")
    gam = nc.dram_tensor("gam", (128, NB * kc), F32, kind="ExternalInput")
    out = nc.dram_tensor("out", (B, nv), F32, kind="ExternalOutput")
    warm = nc.dram_tensor("warm", (128, 128), F32, kind="ExternalOutput")

    xt_ap, w1_ap, w2_ap, gam_ap, out_ap = (
        xt.ap(), w1.ap(), w2.ap(), gam.ap(), out.ap())

    # bcast-muls on GPSIMD except the widest bucket (DVE has slack)
    widest = max(range(len(buckets)), key=lambda i: buckets[i][0] * buckets[i][1])
    gp_buckets = set(range(len(buckets))) - {widest}

    with tile.TileContext(nc) as tc:
        with (
            tc.tile_pool(name="wpool", bufs=1) as wpool,
            tc.tile_pool(name="xpool", bufs=4) as xpool,
            tc.tile_pool(name="ppool", bufs=4, space="PSUM") as ppool,
            tc.tile_pool(name="epool", bufs=5) as epool,
            tc.tile_pool(name="spool", bufs=5) as spool,
            tc.tile_pool(name="opool", bufs=5) as opool,
        ):
            w1t, w2t = [], []
            for r in range(4):
                t = wpool.tile([128, nv], F16, tag=f"w1_{r}")
                nc.sync.dma_start(t[:], w1_ap[r * 128:(r + 1) * 128, :])
                w1t.append(t)
            for r in range(4):
                t = wpool.tile([128, nv], F16, tag=f"w2_{r}")
                nc.sync.dma_start(t[:], w2_ap[r * 128:(r + 1) * 128, :])
                w2t.append(t)
            gall = wpool.tile([128, NB * kc], F32, tag="gam")
            nc.sync.dma_start(gall[:], gam_ap[:, :])

            # HAM warmup: ~5us of dummy matmuls while DMAs land, so the
            # real GEMM starts at 2.4 GHz instead of 1.2
            wz = wpool.tile([128, 128], F16, tag="warmz")
            nc.vector.memset(wz[:], 0.0)
            wps = ppool.tile([128, 1024], F32, tag="ps")
            for i in range(48):
                nc.tensor.matmul(wps[:, 0:128], lhsT=wz[:], rhs=wz[:],
                                 start=True, stop=True)
            wsb = wpool.tile([128, 128], F32, tag="warmsb")
            nc.vector.tensor_copy(wsb[:], wps[:, 0:128])
            nc.sync.dma_start(warm.ap()[:, :], wsb[:])

            for bc in range(NB):
                if bc % 2 == 0:
                    xb2 = xpool.tile([128, 1024], F16, tag="xb")
                    nc.scalar.dma_start(
                        xb2[:].rearrange("p (c r j) -> p c r j", c=2, r=4),
                        xt_ap[bc:bc + 2].rearrange("c r p j -> p c r j"))
                    x2b2 = xpool.tile([128, 1024], F16, tag="x2b")
                    nc.scalar.square(x2b2[:], xb2[:])
                xb = xb2[:, (bc % 2) * 512:(bc % 2) * 512 + 512]
                x2b = x2b2[:, (bc % 2) * 512:(bc % 2) * 512 + 512]

                ps = ppool.tile([128, 1024], F32, tag="ps")
                for c0, cw in colt:
                    for k in range(4):
                        nc.tensor.matmul(
                            ps[:, c0:c0 + cw],
                            lhsT=x2b[:, k * 128:(k + 1) * 128],
                            rhs=w1t[k][:, c0:c0 + cw],
                            start=(k == 0), stop=False,
                        )
                    for k in range(4):
                        nc.tensor.matmul(
                            ps[:, c0:c0 + cw],
                            lhsT=xb[:, k * 128:(k + 1) * 128],
                            rhs=w2t[k][:, c0:c0 + cw],
                            start=False, stop=(k == 3),
                        )

                e = epool.tile([128, nv], F32)
                nc.scalar.activation(e[:], ps[:, 0:nv],
                                     mybir.ActivationFunctionType.Exp)

                ssum = spool.tile([128, kc], F32, tag="ssum")
                off = koff = 0
                for w, n in buckets:
                    nc.vector.reduce_sum(
                        ssum[:, koff:koff + n],
                        e[:, off:off + n * w].rearrange(
                            "p (k m) -> p k m", m=w),
                        axis=mybir.AxisListType.X,
                    )
                    off += n * w
                    koff += n
                rec = spool.tile([128, kc], F32, tag="rec")
                nc.vector.reciprocal_approx_fast(rec[:], ssum[:])
                coef = spool.tile([128, kc], F32, tag="coef")
                nc.vector.tensor_mul(coef[:], rec[:],
                                     gall[:, bc * kc:(bc + 1) * kc])

                o = opool.tile([128, nv], F32)
                off = koff = 0
                for i, (w, n) in enumerate(buckets):
                    e3 = e[:, off:off + n * w].rearrange(
                        "p (k m) -> p k m", m=w)
                    o3 = o[:, off:off + n * w].rearrange(
                        "p (k m) -> p k m", m=w)
                    c3 = coef[:, koff:koff + n].rearrange(
                        "p (k one) -> p k one", one=1)
                    e3b, c3b = bass.broadcast_tensor_aps(e3, c3)
                    eng = nc.gpsimd if i in gp_buckets else nc.vector
                    eng.tensor_tensor(o3, e3b, c3b, op=mybir.AluOpType.mult)
                    off += n * w
                    koff += n
                nc.sync.dma_start(out_ap[bc * 128:(bc + 1) * 128, :], o[:])

    nc.compile()
    return nc


def _layout(mask):
    """Bucket classes by count; round each bucket to a multiple of NCORES by
    promoting classes from the next-lower count pool (cost: 1 wasted column
    each, vs w for a dummy), cascading w=8 down to 2; remaining gaps get
    dummies (-1). Leftover count-1 classes are handled on host (out = gamma
    at mode 0)."""
    counts = mask.sum(-1).astype(int)               # (K,)
    pools = {c: list(np.where(counts == c)[0]) for c in range(1, M + 1)}
    entries = []                                    # (w, global id list)
    for w in range(M, 1, -1):
        ids = pools[w]
        pools[w] = []
        pad = (-len(ids)) % NCORES
        if pad and len(pools[w - 1]) >= pad:
            ids += pools[w - 1][:pad]
            pools[w - 1] = pools[w - 1][pad:]
        elif pad:
            ids += [-1] * pad
        if ids:
            entries.append((w, ids))
    entries.sort()
    per_core = [[] for _ in range(NCORES)]
    buckets = []
    for w, ids in entries:
        n = len(ids) // NCORES
        buckets.append((w, n))
        for c in range(NCORES):
            per_core[c].append((w, ids[c * n:(c + 1) * n]))
    ones = np.asarray(pools[1], int)
    return tuple(buckets), per_core, ones


def prep_inputs(x, gamma_class, mu_pad, var_pad, pi_pad, mask):
    x = np.asarray(x, np.float32)
    gamma_class = np.asarray(gamma_class, np.float32)
    mask = np.asarray(mask, bool)

    var = np.clip(np.asarray(var_pad, np.float64) + EPS_REG, 1e-8, None)
    inv = 1.0 / var
    logdet = np.log(var).sum(-1)                      # (K, M)
    muinv = np.asarray(mu_pad, np.float64) * inv
    muinvmu = (np.asarray(mu_pad, np.float64) * muinv).sum(-1)
    logpi = np.where(mask, np.log(np.asarray(pi_pad, np.float64) + 1e-10),
                     -np.inf)
    u = -0.5 * logdet + logpi
    c = u.max(axis=1)
    carry = np.maximum(u - 0.5 * muinvmu - c[:, None], -20000.0)  # (K, M)

    w1_full = (-0.5 * inv + carry[..., None]).astype(np.float32)  # (K, M, D)
    w2_full = muinv.astype(np.float32)

    buckets, per_core, ones = _layout(mask)
    nv = sum(w * n for w, n in buckets)
    kc = sum(n for _, n in buckets)

    xtb = np.ascontiguousarray(
        x.reshape(NB, 128, 4, 128).transpose(0, 2, 3, 1).astype(np.float16))

    in_maps = []
    for cidx in range(NCORES):
        w1c = np.zeros((nv, D), np.float32)
        w2c = np.zeros((nv, D), np.float32)
        gcols = np.zeros((B, kc), np.float32)
        off = koff = 0
        for w, ids in per_core[cidx]:
            for k in ids:
                if k >= 0:
                    w1c[off:off + w] = w1_full[k, :w]
                    w2c[off:off + w] = w2_full[k, :w]
                    gcols[:, koff] = gamma_class[:, k]
                off += w
                koff += 1
        in_maps.append({
            "xt": xtb,
            "w1": np.ascontiguousarray(w1c.T.astype(np.float16)),
            "w2": np.ascontiguousarray(w2c.T.astype(np.float16)),
            "gam": np.ascontiguousarray(
                gcols.reshape(NB, 128, kc).transpose(1, 0, 2)
                .reshape(128, NB * kc)),
        })
    return in_maps, buckets, per_core, ones


_NC_CACHE = {}


def _get_nc(buckets):
    if buckets not in _NC_CACHE:
        _NC_CACHE[buckets] = build_bass(buckets)
    return _NC_CACHE[buckets]


def kernel(x, gamma_class, mu_pad, var_pad, pi_pad, mask, _trace=False):
    in_maps, buckets, per_core, ones = prep_inputs(
        x, gamma_class, mu_pad, var_pad, pi_pad, mask)
    if not buckets:  # every class has a single mode: softmax is identically 1
        out = np.zeros((B, K, M), np.float32)
        out[:, ones, 0] = np.asarray(gamma_class, np.float32)[:, ones]
        return out
    nc = _get_nc(buckets)
    res = bass_utils.run_bass_kernel_spmd(
        nc, in_maps, core_ids=list(range(NCORES)), trace=_trace)

    counts = np.asarray(mask, bool).sum(-1).astype(int)
    out = np.zeros((B, K, M), np.float32)
    gamma_class = np.asarray(gamma_class, np.float32)
    if len(ones):
        out[:, ones, 0] = gamma_class[:, ones]
    for cidx in range(NCORES):
        packed = res.results[cidx]["out"]             # (B, nv)
        off = 0
        for w, ids in per_core[cidx]:
            ids = np.asarray(ids)
            blk = packed[:, off:off + len(ids) * w].reshape(B, len(ids), w)
            real = ids >= 0
            # promoted classes have count < w: scatter only their real modes
            for c in np.unique(counts[ids[real]]):
                sel = real & (counts[np.maximum(ids, 0)] == c)
                out[:, ids[sel], :c] = blk[:, sel, :c]
            off += len(ids) * w
    if _trace:
        kernel.last_results = res
    return out
